# revision 51
# baseline (speedup 1.0000x reference)
"""Trainium2 Bass kernel for a 2-layer ReLU-RNN (ABDRNN).

Math (per layer): wx = x @ W^T + b ; h_t = relu(r*h_{t-1} + (1-r)*wx_t)

Strategy:
- Data-parallel over batch M (64) across 8 NeuronCores (M_local=8 each).
- Per core, time is chunked (Tc steps). For each chunk:
    GEMM (TensorE, bf16 inputs, f32 PSUM accumulation) computes
      u~_t = (1-r) * (x_t @ W^T + b) * r^{-(t'+1)}
    with the (1-r)*r^{-(t'+1)} scale folded into x host-side and the bias
    added via a K=1 matmul against a precomputed scale row.
- The ReLU recurrence is computed exactly with the VectorE hardware scan:
      relu(r*h + u) == r^{t'+1} * g_t,  g_t = max(g_{t-1} + u~_t, 0)
  i.e. tensor_tensor_scan(op0=add, op1=max) in a per-chunk rescaled space,
  with per-(m) carries chained through a [128, ht*m] SBUF carry tile.
- Layer-0 scan output g0 is multiplied by rho_t = (1-r1)*(r0/r1)^{t'+1}
  (GpSimdE) to become layer-1's pre-scaled bf16 GEMM input directly.
- Layer-1 g1 is DMA'd out in scaled space; the host multiplies by r1^{t'+1}
  and restores [T, M, H] layout. h_n[0] comes from the layer-0 carry tile
  (true h); h_n[1] = all_h1[-1].
"""
import sys
import math

sys.path.insert(0, "/opt/trn_rl_repo")

import numpy as np
import ml_dtypes
from contextlib import ExitStack

BF = ml_dtypes.bfloat16

T, M, I, H, NCORES = 512, 64, 1024, 1024, 8
ML = M // NCORES  # 8 batch rows per core
KI = I // 128     # 8 contraction chunks
KH = H // 128     # 8 output h-tiles

_CACHE = {}


def _pick_tc(r0, r1):
    ratios = [1.0 / r0, 1.0 / r1, r0 / r1, r1 / r0]
    lr = max(math.log(max(ratios)), 1e-12)
    tc_max = 60.0 / lr  # keep |scale| <= ~1e26
    for tc in (256, 128, 64, 32, 16, 8):
        if tc <= tc_max:
            return tc
    raise NotImplementedError(f"rgate values too extreme: r0={r0} r1={r1}")


def _build(r0, r1, Tc):
    import concourse.mybir as mybir
    import concourse.tile as tile
    from concourse import bacc

    f32 = mybir.dt.float32
    bf16 = mybir.dt.bfloat16
    AD, MX = mybir.AluOpType.add, mybir.AluOpType.max

    C = T // Tc
    FREE = ML * Tc                 # free extent of one (m, t') chunk
    FCSZ = min(512, FREE)          # free-chunk per matmul (PSUM bank limit)
    NFC = FREE // FCSZ
    MPF = FCSZ // Tc               # m-sequences per free chunk

    nc = bacc.Bacc("TRN2", target_bir_lowering=False, debug=False,
                   enable_asserts=False, num_devices=NCORES)

    xd = nc.dram_tensor("x", [KI, C, 128, ML, Tc], bf16, kind="ExternalInput").ap()
    wd = [nc.dram_tensor(f"w{l}", [128, KI * KH * 128], bf16, kind="ExternalInput").ap()
          for l in range(2)]
    btd = [nc.dram_tensor(f"bt{l}", [128, KH * Tc], f32, kind="ExternalInput").ap()
           for l in range(2)]
    ccd = nc.dram_tensor("cc", [128, 2, KH], f32, kind="ExternalInput").ap()
    rhod = nc.dram_tensor("rho", [128, FREE], f32, kind="ExternalInput").ap()
    h0d = nc.dram_tensor("h0T", [2, 128, KH, ML], f32, kind="ExternalInput").ap()
    outg = nc.dram_tensor("outg", [C, KH, 128, ML, Tc], f32, kind="ExternalOutput").ap()
    outh = nc.dram_tensor("outhn0", [128, KH, ML], f32, kind="ExternalOutput").ap()

    with tile.TileContext(nc) as tc, ExitStack() as ctx:
        consts = ctx.enter_context(tc.tile_pool(name="consts", bufs=1))
        # x tiles: ~44KB/partition of buffering (whole chunk + most of next)
        xbufs = min(C * KI, max(KI + 1, (36 * 1024) // (FREE * 2)))
        xpool = ctx.enter_context(tc.tile_pool(name="xin", bufs=xbufs))
        pspool = ctx.enter_context(tc.tile_pool(name="ps", bufs=8, space="PSUM"))
        g0pool = ctx.enter_context(tc.tile_pool(name="g0", bufs=3))
        x1pool = ctx.enter_context(tc.tile_pool(name="x1", bufs=C * KI))
        g1pool = ctx.enter_context(tc.tile_pool(name="g1", bufs=3))

        # DMA emission order tuned so layer-0's first MM group can start
        # ~2us in: bias/srow first, then x(0,k) + w0 k-slices interleaved.
        wsb = [consts.tile([128, KI * KH * 128], bf16, tag=f"w{l}", name=f"w{l}sb")
               for l in range(2)]
        btsb = [consts.tile([128, KH * Tc], f32, tag=f"bt{l}", name=f"bt{l}sb")
                for l in range(2)]
        ccsb = consts.tile([128, 2, KH], f32, tag="cc", name="ccsb")
        nc.sync.dma_start(ccsb[:], ccd)
        car = [consts.tile([128, KH * ML], f32, tag=f"car{l}", name=f"car{l}")
               for l in range(2)]
        nc.sync.dma_start(car[0][:], h0d[0].rearrange("p ht m -> p (ht m)"))

        xt = {}   # (c, k) -> bf16 tile [128, FREE]
        KW = KH * 128
        HF = FREE // 2
        for k in range(KI):
            t_ = xpool.tile([128, FREE], bf16, tag="xt", name="xt")
            nc.sync.dma_start(t_[:, :HF],
                              xd[k, 0].rearrange("p m t -> p (m t)")[:, :HF])
            xt[(0, k)] = t_
            nc.sync.dma_start(wsb[0][:, k * KW:(k + 1) * KW],
                              wd[0][:, k * KW:(k + 1) * KW])
        for k in range(KI):
            nc.sync.dma_start(xt[(0, k)][:, HF:],
                              xd[k, 0].rearrange("p m t -> p (m t)")[:, HF:])
        # layer-1 / later-chunk constants stream in behind
        nc.sync.dma_start(btsb[0][:], btd[0])
        nc.sync.dma_start(btsb[1][:], btd[1])
        nc.sync.dma_start(car[1][:], h0d[1].rearrange("p ht m -> p (ht m)"))
        rsb = consts.tile([128, FREE], f32, tag="rho")
        nc.sync.dma_start(rsb[:], rhod)
        for k in range(KI):
            nc.sync.dma_start(wsb[1][:, k * KW:(k + 1) * KW],
                              wd[1][:, k * KW:(k + 1) * KW])
        for c in range(1, C):
            for k in range(KI):
                t_ = xpool.tile([128, FREE], bf16, tag="xt", name="xt")
                nc.sync.dma_start(t_[:], xd[k, c].rearrange("p m t -> p (m t)"))
                xt[(c, k)] = t_

        x1t = {}  # (c, k) -> bf16 tile (layer-1 rhs)
        psums = {}

        def gemm(l, c, fc_sets=None):
            # fc-interleaved so each stationary weight load serves the fc set
            if fc_sets is None:
                fc_sets = [tuple(range(NFC))]
            for fcs in fc_sets:
                for ht in range(KH):
                    pss = {fc: pspool.tile([128, FCSZ], f32, tag="ps", name="ps")
                           for fc in fcs}
                    for k in range(KI):
                        for fc in fcs:
                            rhs_t = xt[(c, k)] if l == 0 else x1t[(c, k)]
                            nc.tensor.matmul(
                                pss[fc][:],
                                lhsT=wsb[l][:, (k * KH + ht) * 128:(k * KH + ht + 1) * 128],
                                rhs=rhs_t[:, fc * FCSZ:(fc + 1) * FCSZ],
                                start=(k == 0), stop=(k == KI - 1))
                    for fc in fcs:
                        psums[(l, c, ht, fc)] = pss[fc]

        def scan(l, c):
            r = r0 if l == 0 else r1
            rTc = float(r) ** Tc
            for ht in range(KH):
                g = (g0pool if l == 0 else g1pool).tile([128, FREE], f32)
                for fc in range(NFC):
                    ps = psums.pop((l, c, ht, fc))
                    for mm in range(MPF):
                        m = fc * MPF + mm
                        nc.vector.tensor_tensor_scan(
                            out=g[:, m * Tc:(m + 1) * Tc],
                            data0=ps[:, mm * Tc:(mm + 1) * Tc],
                            data1=btsb[l][:, ht * Tc:(ht + 1) * Tc],
                            initial=car[l][:, ht * ML + m:ht * ML + m + 1],
                            op0=AD, op1=MX)
                gv = g[:].rearrange("p (m t) -> p m t", t=Tc)
                nc.gpsimd.tensor_scalar(
                    car[l][:, ht * ML:(ht + 1) * ML], gv[:, :, Tc - 1:Tc], rTc,
                    ccsb[:, l, ht:ht + 1], mybir.AluOpType.mult, AD)
                if l == 0:
                    x1 = x1pool.tile([128, FREE], bf16)
                    nc.gpsimd.tensor_mul(x1[:], g[:], rsb[:])
                    x1t[(c, ht)] = x1
                else:
                    ov = outg[c, ht].rearrange("p m t -> p (m t)")
                    nc.sync.dma_start(ov[:, :FREE // 2], g[:, :FREE // 2])
                    nc.sync.dma_start(ov[:, FREE // 2:], g[:, FREE // 2:])

        # Layer-sequential emission: per-engine streams trail each other
        # (scans chase psums ht-by-ht), so layer 1's GEMM starts right after
        # layer 0's last GEMM with x1 tiles already materialized.
        def l0c0_halves():
            # chunk-0 layer-0 in two m-half passes: the first pass needs only
            # half the x bytes, so PE starts sooner during the input stream.
            h_ = NFC // 2
            rTc = float(r0) ** Tc
            mh = ML // 2
            for hi, fcs in enumerate((tuple(range(h_)), tuple(range(h_, NFC)))):
                # pass 1 runs k-outer in waves of 4 h-tiles so PE consumes
                # each (x,w) pair as it arrives from HBM
                waves = ((0, 1, 2, 3), (4, 5, 6, 7)) if hi == 0 else                     (tuple(range(KH)),)
                wavepss = {}
                for wave in waves:
                    if hi == 0:
                        for ht in wave:
                            wavepss[ht] = {fc: pspool.tile([128, FCSZ], f32,
                                                           tag="ps", name="ps")
                                           for fc in fcs}
                        for k in range(KI):
                            for ht in wave:
                                for fc in fcs:
                                    nc.tensor.matmul(
                                        wavepss[ht][fc][:],
                                        lhsT=wsb[0][:, (k * KH + ht) * 128:(k * KH + ht + 1) * 128],
                                        rhs=xt[(0, k)][:, fc * FCSZ:(fc + 1) * FCSZ],
                                        start=(k == 0), stop=(k == KI - 1))
                for ht in range(KH):
                    if hi == 0:
                        pss = wavepss[ht]
                    else:
                        pss = {fc: pspool.tile([128, FCSZ], f32, tag="ps", name="ps")
                               for fc in fcs}
                        for k in range(KI):
                            for fc in fcs:
                                nc.tensor.matmul(
                                    pss[fc][:],
                                    lhsT=wsb[0][:, (k * KH + ht) * 128:(k * KH + ht + 1) * 128],
                                    rhs=xt[(0, k)][:, fc * FCSZ:(fc + 1) * FCSZ],
                                    start=(k == 0), stop=(k == KI - 1))
                    if hi == 0:
                        x1t[(0, ht)] = x1pool.tile([128, FREE], bf16,
                                                   tag="x1", name="x1")
                    x1 = x1t[(0, ht)]
                    g = g0pool.tile([128, HF], f32, tag="g", name="g")
                    for j, fc in enumerate(fcs):
                        ps = pss[fc]
                        for mm in range(MPF):
                            m = fc * MPF + mm
                            lm = j * MPF + mm
                            nc.vector.tensor_tensor_scan(
                                out=g[:, lm * Tc:(lm + 1) * Tc],
                                data0=ps[:, mm * Tc:(mm + 1) * Tc],
                                data1=btsb[0][:, ht * Tc:(ht + 1) * Tc],
                                initial=car[0][:, ht * ML + m:ht * ML + m + 1],
                                op0=AD, op1=MX)
                    gv = g[:].rearrange("p (m t) -> p m t", t=Tc)
                    nc.gpsimd.tensor_scalar(
                        car[0][:, ht * ML + hi * mh:ht * ML + (hi + 1) * mh],
                        gv[:, :, Tc - 1:Tc], rTc, ccsb[:, 0, ht:ht + 1],
                        mybir.AluOpType.mult, AD)
                    nc.gpsimd.tensor_mul(
                        x1[:, hi * HF:(hi + 1) * HF], g[:],
                        rsb[:, hi * HF:(hi + 1) * HF])

        for c in range(C):
            if c == 0 and NFC >= 2 and ML % 2 == 0:
                l0c0_halves()
            else:
                gemm(0, c)
                scan(0, c)
        nc.sync.dma_start(outh.rearrange("p ht m -> p (ht m)"), car[0][:])

        for c in range(C):
            gemm(1, c)
            scan(1, c)

    nc.compile()
    return nc


def _prep_core(x_core, h0_core, scale0):
    """x_core: [T, ML, I] f32; returns bf16 x in [KI, C, 128, ML, Tc] layout."""
    Tc = scale0.shape[0] // 1
    xs = (x_core.astype(np.float32)
          * np.tile(scale0, T // len(scale0))[:, None, None]).astype(BF)
    # [T, ML, I] -> [I, ML, T] -> [KI, 128, ML, C, Tc] -> [KI, C, 128, ML, Tc]
    C = T // Tc
    xp = xs.transpose(2, 1, 0).reshape(KI, 128, ML, C, Tc).transpose(0, 3, 1, 2, 4)
    # h0T[l, p, ht, m] = h_0[l, m, ht*128 + p]
    h0T = (h0_core.transpose(0, 2, 1).reshape(2, KH, 128, ML)
           .transpose(0, 2, 1, 3)).astype(np.float32)
    return np.ascontiguousarray(xp), np.ascontiguousarray(h0T)


def kernel(input_, h_0, w_ih0, b_ih0, rgate0, w_ih1, b_ih1, rgate1):
    from concourse.bass_utils import run_bass_kernel_spmd

    r0 = float(np.asarray(rgate0)); r1 = float(np.asarray(rgate1))
    # Clamp degenerate gates: for r below ~1e-3 the recurrence term r*h is
    # negligible (error of the clamp is O(1e-3) relative), but the rescaled
    # max-plus scan needs r^(-Tc) to stay within f32 range.
    r0 = min(max(r0, 1e-3), 1e3)
    r1 = min(max(r1, 1e-3), 1e3)
    Tc = _pick_tc(r0, r1)
    C = T // Tc
    key = (round(r0, 9), round(r1, 9), Tc)
    if key not in _CACHE:
        _CACHE[key] = _build(r0, r1, Tc)
    nc = _CACHE[key]

    FREE = ML * Tc
    FCSZ = min(512, FREE)
    MPF = FCSZ // Tc
    tp = np.arange(Tc, dtype=np.float64)
    scale0 = ((1.0 - r0) * r0 ** -(tp + 1)).astype(np.float32)  # [Tc]
    rho64 = (1.0 - r1) * (r0 / r1) ** (tp + 1)
    rho_full = np.ascontiguousarray(
        np.broadcast_to(rho64.astype(np.float32), (128, ML, Tc))
        .reshape(128, ML * Tc))

    def prep_w(w):
        # [H, I] -> [p, ic, ht, hh] -> [128, KI*KH*128]
        return np.ascontiguousarray(
            np.asarray(w, np.float32).astype(BF)
            .reshape(KH, 128, KI, 128).transpose(3, 2, 0, 1).reshape(128, -1))

    w0p, w1p = prep_w(w_ih0), prep_w(w_ih1)
    # Bias via the scan's data1 operand: shift y_t = g_t - B_t turns the bias
    # add into data1 = -B_t (a constant). Layer-1 additionally absorbs the
    # bias-through-layer-0 term (rank-1: q = W1*b0) via the same shift.
    b0v = np.asarray(b_ih0, np.float64).ravel()
    b1v = np.asarray(b_ih1, np.float64).ravel()
    S0 = np.cumsum((1.0 - r0) * r0 ** -(tp + 1))          # [Tc]
    S1 = np.cumsum((1.0 - r1) * r1 ** -(tp + 1))
    w1b = np.asarray(w_ih1, np.float32).astype(BF).astype(np.float64)
    qv = w1b @ b0v                                        # [H]
    Pc = np.cumsum(rho64 * S0)                            # [Tc]
    B0g = b0v[None, :] * S0[:, None]                      # [Tc, H]
    corr1 = b1v[None, :] * S1[:, None] + qv[None, :] * Pc[:, None]  # [Tc, H]

    def to_bt(mat):  # [Tc, H] -> [128, KH*Tc] (p, ht, t)
        return np.ascontiguousarray(
            (-mat).T.reshape(KH, 128, Tc).transpose(1, 0, 2)
            .reshape(128, KH * Tc).astype(np.float32))

    bt0 = to_bt(B0g)
    bt1 = to_bt(corr1)
    ccp = np.empty((128, 2, KH), np.float32)
    ccp[:, 0, :] = (B0g[Tc - 1] * r0 ** Tc).reshape(KH, 128).T
    ccp[:, 1, :] = (corr1[Tc - 1] * r1 ** Tc).reshape(KH, 128).T

    input_ = np.asarray(input_, np.float32)
    h_0 = np.asarray(h_0, np.float32)

    in_maps = []
    for core in range(NCORES):
        xc = input_[:, core * ML:(core + 1) * ML, :]
        hc = h_0[:, core * ML:(core + 1) * ML, :]
        xp, h0T = _prep_core(xc, hc, scale0)
        in_maps.append({
            "x": xp, "w0": w0p, "w1": w1p,
            "bt0": bt0, "bt1": bt1, "cc": ccp,
            "rho": rho_full, "h0T": h0T,
        })

    res = run_bass_kernel_spmd(nc, in_maps, core_ids=list(range(NCORES)))
    results = res.results

    rpow1 = (r1 ** (tp + 1)).astype(np.float32)  # [Tc]
    corr_dev = corr1.T.reshape(KH, 128, Tc).astype(np.float32)  # [ht, p, t]
    all_h1 = np.empty((T, M, H), np.float32)
    h_n = np.empty((2, M, H), np.float32)
    for core in range(NCORES):
        g = results[core]["outg"].reshape(C, KH, 128, ML, Tc)
        g = g + corr_dev[None, :, :, None, :]
        hcore = (g * rpow1[None, None, None, None, :]) \
            .transpose(0, 4, 3, 1, 2).reshape(T, ML, H)
        all_h1[:, core * ML:(core + 1) * ML, :] = hcore
        hn0 = results[core]["outhn0"].reshape(128, KH, ML) \
            .transpose(2, 1, 0).reshape(ML, H)
        h_n[0, core * ML:(core + 1) * ML, :] = hn0
    h_n[1] = all_h1[T - 1]
    return all_h1, h_n


# revision 53
# speedup vs baseline: 1.2160x; 1.2160x over previous
"""Trainium2 Bass kernel for a 2-layer ReLU-RNN (ABDRNN).

Math (per layer): wx = x @ W^T + b ; h_t = relu(r*h_{t-1} + (1-r)*wx_t)

Strategy:
- Data-parallel over batch M (64) across 8 NeuronCores (M_local=8 each).
- Per core, time is chunked (Tc steps). For each chunk:
    GEMM (TensorE, bf16 inputs, f32 PSUM accumulation) computes
      u~mm_t = (1-r) * (x_t @ W^T) * r^{-(t'+1)}
    with the (1-r)*r^{-(t'+1)} scale folded into x host-side.
- The ReLU recurrence is computed exactly with the VectorE hardware scan
  in a per-chunk rescaled space, with the bias riding in the scan's data1
  operand: shifting by the bias prefix-sum B_t turns
      g_t = max(g_{t-1} + u~mm_t + b~_t, 0)   into
      y_t = max(y_{t-1} + u~mm_t, -B_t)       (g = y + B, B precomputed)
  i.e. tensor_tensor_scan(op0=add, op1=max) with data1 = -B_t constants;
  layer 1's bias-through-layer-0 term is rank-1 (q = W1@b0) and folds into
  its data1 table and the host-side un-scaling. Per-(m) carries chain
  through a [128, ht*m] SBUF carry tile (tensor_scalar mult+add restores
  true-h carries).
- Layer-0 scan output g0 is multiplied by rho_t = (1-r1)*(r0/r1)^{t'+1}
  (GpSimdE) to become layer-1's pre-scaled bf16 GEMM input directly.
- Layer-1 g1 is DMA'd out in scaled space; the host multiplies by r1^{t'+1}
  and restores [T, M, H] layout. h_n[0] comes from the layer-0 carry tile
  (true h); h_n[1] = all_h1[-1].
"""
import sys
import math

sys.path.insert(0, "/opt/trn_rl_repo")

import numpy as np
import ml_dtypes
from contextlib import ExitStack

BF = ml_dtypes.bfloat16

T, M, I, H, NCORES = 512, 64, 1024, 1024, 8
ML = M // NCORES  # 8 batch rows per core
KI = I // 128     # 8 contraction chunks
KH = H // 128     # 8 output h-tiles

_CACHE = {}


def _pick_tc(r0, r1):
    ratios = [1.0 / r0, 1.0 / r1, r0 / r1, r1 / r0]
    lr = max(math.log(max(ratios)), 1e-12)
    tc_max = 60.0 / lr  # keep |scale| <= ~1e26
    for tc in (256, 128, 64, 32, 16, 8):
        if tc <= tc_max:
            return tc
    raise NotImplementedError(f"rgate values too extreme: r0={r0} r1={r1}")


def _build(r0, r1, Tc):
    import concourse.mybir as mybir
    import concourse.tile as tile
    from concourse import bacc

    f32 = mybir.dt.float32
    bf16 = mybir.dt.bfloat16
    AD, MX = mybir.AluOpType.add, mybir.AluOpType.max

    C = T // Tc
    FREE = ML * Tc                 # free extent of one (m, t') chunk
    FCSZ = min(512, FREE)          # free-chunk per matmul (PSUM bank limit)
    NFC = FREE // FCSZ
    MPF = FCSZ // Tc               # m-sequences per free chunk

    nc = bacc.Bacc("TRN2", target_bir_lowering=False, debug=False,
                   enable_asserts=False, num_devices=NCORES)

    xd = nc.dram_tensor("x", [KI, C, 128, ML, Tc], bf16, kind="ExternalInput").ap()
    wd = [nc.dram_tensor(f"w{l}", [128, KI * KH * 128], bf16, kind="ExternalInput").ap()
          for l in range(2)]
    btd = [nc.dram_tensor(f"bt{l}", [128, KH * Tc], f32, kind="ExternalInput").ap()
           for l in range(2)]
    ccd = nc.dram_tensor("cc", [128, 2, KH], f32, kind="ExternalInput").ap()
    rhod = nc.dram_tensor("rho", [128, FREE], f32, kind="ExternalInput").ap()
    h0d = nc.dram_tensor("h0T", [2, 128, KH, ML], f32, kind="ExternalInput").ap()
    outg = nc.dram_tensor("outg", [C, KH, 128, ML, Tc], f32, kind="ExternalOutput").ap()
    outh = nc.dram_tensor("outhn0", [128, KH, ML], f32, kind="ExternalOutput").ap()

    with tile.TileContext(nc) as tc, ExitStack() as ctx:
        consts = ctx.enter_context(tc.tile_pool(name="consts", bufs=1))
        # x tiles: ~44KB/partition of buffering (whole chunk + most of next)
        xbufs = min(C * KI, max(KI + 1, (36 * 1024) // (FREE * 2)))
        xpool = ctx.enter_context(tc.tile_pool(name="xin", bufs=xbufs))
        pspool = ctx.enter_context(tc.tile_pool(name="ps", bufs=8, space="PSUM"))
        g0pool = ctx.enter_context(tc.tile_pool(name="g0", bufs=3))
        x1pool = ctx.enter_context(tc.tile_pool(name="x1", bufs=C * KI))
        g1pool = ctx.enter_context(tc.tile_pool(name="g1", bufs=3))

        # DMA emission order tuned so layer-0's first MM group can start
        # ~2us in: bias/srow first, then x(0,k) + w0 k-slices interleaved.
        wsb = [consts.tile([128, KI * KH * 128], bf16, tag=f"w{l}", name=f"w{l}sb")
               for l in range(2)]
        btsb = [consts.tile([128, KH * Tc], f32, tag=f"bt{l}", name=f"bt{l}sb")
                for l in range(2)]
        ccsb = consts.tile([128, 2, KH], f32, tag="cc", name="ccsb")
        nc.sync.dma_start(ccsb[:], ccd)
        car = [consts.tile([128, KH * ML], f32, tag=f"car{l}", name=f"car{l}")
               for l in range(2)]
        nc.sync.dma_start(car[0][:], h0d[0].rearrange("p ht m -> p (ht m)"))

        xt = {}   # (c, k) -> bf16 tile [128, FREE]
        KW = KH * 128
        HF = FREE // 2
        for k in range(KI):
            t_ = xpool.tile([128, FREE], bf16, tag="xt", name="xt")
            nc.sync.dma_start(t_[:, :HF],
                              xd[k, 0].rearrange("p m t -> p (m t)")[:, :HF])
            xt[(0, k)] = t_
            nc.sync.dma_start(wsb[0][:, k * KW:(k + 1) * KW],
                              wd[0][:, k * KW:(k + 1) * KW])
        for k in range(KI):
            nc.sync.dma_start(xt[(0, k)][:, HF:],
                              xd[k, 0].rearrange("p m t -> p (m t)")[:, HF:])
        # layer-1 / later-chunk constants stream in behind
        nc.sync.dma_start(btsb[0][:], btd[0])
        nc.sync.dma_start(btsb[1][:], btd[1])
        nc.sync.dma_start(car[1][:], h0d[1].rearrange("p ht m -> p (ht m)"))
        rsb = consts.tile([128, FREE], f32, tag="rho")
        nc.sync.dma_start(rsb[:], rhod)
        for k in range(KI):
            nc.sync.dma_start(wsb[1][:, k * KW:(k + 1) * KW],
                              wd[1][:, k * KW:(k + 1) * KW])
        for c in range(1, C):
            for k in range(KI):
                t_ = xpool.tile([128, FREE], bf16, tag="xt", name="xt")
                nc.sync.dma_start(t_[:], xd[k, c].rearrange("p m t -> p (m t)"))
                xt[(c, k)] = t_

        x1t = {}  # (c, k) -> bf16 tile (layer-1 rhs)
        psums = {}

        def gemm(l, c, fc_sets=None):
            # fc-interleaved so each stationary weight load serves the fc set
            if fc_sets is None:
                fc_sets = [tuple(range(NFC))]
            for fcs in fc_sets:
                for ht in range(KH):
                    pss = {fc: pspool.tile([128, FCSZ], f32, tag="ps", name="ps")
                           for fc in fcs}
                    for k in range(KI):
                        for fc in fcs:
                            rhs_t = xt[(c, k)] if l == 0 else x1t[(c, k)]
                            nc.tensor.matmul(
                                pss[fc][:],
                                lhsT=wsb[l][:, (k * KH + ht) * 128:(k * KH + ht + 1) * 128],
                                rhs=rhs_t[:, fc * FCSZ:(fc + 1) * FCSZ],
                                start=(k == 0), stop=(k == KI - 1))
                    for fc in fcs:
                        psums[(l, c, ht, fc)] = pss[fc]

        def scan(l, c):
            r = r0 if l == 0 else r1
            rTc = float(r) ** Tc
            for ht in range(KH):
                g = (g0pool if l == 0 else g1pool).tile([128, FREE], f32)
                for fc in range(NFC):
                    ps = psums.pop((l, c, ht, fc))
                    for mm in range(MPF):
                        m = fc * MPF + mm
                        nc.vector.tensor_tensor_scan(
                            out=g[:, m * Tc:(m + 1) * Tc],
                            data0=ps[:, mm * Tc:(mm + 1) * Tc],
                            data1=btsb[l][:, ht * Tc:(ht + 1) * Tc],
                            initial=car[l][:, ht * ML + m:ht * ML + m + 1],
                            op0=AD, op1=MX)
                gv = g[:].rearrange("p (m t) -> p m t", t=Tc)
                nc.gpsimd.tensor_scalar(
                    car[l][:, ht * ML:(ht + 1) * ML], gv[:, :, Tc - 1:Tc], rTc,
                    ccsb[:, l, ht:ht + 1], mybir.AluOpType.mult, AD)
                if l == 0:
                    x1 = x1pool.tile([128, FREE], bf16)
                    nc.gpsimd.tensor_mul(x1[:], g[:], rsb[:])
                    x1t[(c, ht)] = x1
                else:
                    ov = outg[c, ht].rearrange("p m t -> p (m t)")
                    nc.sync.dma_start(ov[:, :FREE // 2], g[:, :FREE // 2])
                    nc.sync.dma_start(ov[:, FREE // 2:], g[:, FREE // 2:])

        # Layer-sequential emission: per-engine streams trail each other
        # (scans chase psums ht-by-ht), so layer 1's GEMM starts right after
        # layer 0's last GEMM with x1 tiles already materialized.
        def l0c0_halves():
            # chunk-0 layer-0 in two m-half passes: the first pass needs only
            # half the x bytes, so PE starts sooner during the input stream.
            h_ = NFC // 2
            rTc = float(r0) ** Tc
            mh = ML // 2
            for hi, fcs in enumerate((tuple(range(h_)), tuple(range(h_, NFC)))):
                for ht in range(KH):
                    pss = {fc: pspool.tile([128, FCSZ], f32, tag="ps", name="ps")
                           for fc in fcs}
                    for k in range(KI):
                        for fc in fcs:
                            nc.tensor.matmul(
                                pss[fc][:],
                                lhsT=wsb[0][:, (k * KH + ht) * 128:(k * KH + ht + 1) * 128],
                                rhs=xt[(0, k)][:, fc * FCSZ:(fc + 1) * FCSZ],
                                start=(k == 0), stop=(k == KI - 1))
                    if hi == 0:
                        x1t[(0, ht)] = x1pool.tile([128, FREE], bf16,
                                                   tag="x1", name="x1")
                    x1 = x1t[(0, ht)]
                    g = g0pool.tile([128, HF], f32, tag="g", name="g")
                    for j, fc in enumerate(fcs):
                        ps = pss[fc]
                        for mm in range(MPF):
                            m = fc * MPF + mm
                            lm = j * MPF + mm
                            nc.vector.tensor_tensor_scan(
                                out=g[:, lm * Tc:(lm + 1) * Tc],
                                data0=ps[:, mm * Tc:(mm + 1) * Tc],
                                data1=btsb[0][:, ht * Tc:(ht + 1) * Tc],
                                initial=car[0][:, ht * ML + m:ht * ML + m + 1],
                                op0=AD, op1=MX)
                    gv = g[:].rearrange("p (m t) -> p m t", t=Tc)
                    nc.gpsimd.tensor_scalar(
                        car[0][:, ht * ML + hi * mh:ht * ML + (hi + 1) * mh],
                        gv[:, :, Tc - 1:Tc], rTc, ccsb[:, 0, ht:ht + 1],
                        mybir.AluOpType.mult, AD)
                    nc.gpsimd.tensor_mul(
                        x1[:, hi * HF:(hi + 1) * HF], g[:],
                        rsb[:, hi * HF:(hi + 1) * HF])

        for c in range(C):
            if c == 0 and NFC >= 2 and ML % 2 == 0:
                l0c0_halves()
            else:
                gemm(0, c)
                scan(0, c)
        nc.sync.dma_start(outh.rearrange("p ht m -> p (ht m)"), car[0][:])

        for c in range(C):
            gemm(1, c)
            scan(1, c)

    nc.compile()
    return nc


def _prep_core(x_core, h0_core, scale0):
    """x_core: [T, ML, I] f32; returns bf16 x in [KI, C, 128, ML, Tc] layout."""
    Tc = scale0.shape[0] // 1
    xs = (x_core.astype(np.float32)
          * np.tile(scale0, T // len(scale0))[:, None, None]).astype(BF)
    # [T, ML, I] -> [I, ML, T] -> [KI, 128, ML, C, Tc] -> [KI, C, 128, ML, Tc]
    C = T // Tc
    xp = xs.transpose(2, 1, 0).reshape(KI, 128, ML, C, Tc).transpose(0, 3, 1, 2, 4)
    # h0T[l, p, ht, m] = h_0[l, m, ht*128 + p]
    h0T = (h0_core.transpose(0, 2, 1).reshape(2, KH, 128, ML)
           .transpose(0, 2, 1, 3)).astype(np.float32)
    return np.ascontiguousarray(xp), np.ascontiguousarray(h0T)


def kernel(input_, h_0, w_ih0, b_ih0, rgate0, w_ih1, b_ih1, rgate1):
    from concourse.bass_utils import run_bass_kernel_spmd

    r0 = float(np.asarray(rgate0)); r1 = float(np.asarray(rgate1))
    # Clamp degenerate gates: for r below ~1e-3 the recurrence term r*h is
    # negligible (error of the clamp is O(1e-3) relative), but the rescaled
    # max-plus scan needs r^(-Tc) to stay within f32 range.
    r0 = min(max(r0, 1e-3), 1e3)
    r1 = min(max(r1, 1e-3), 1e3)
    Tc = _pick_tc(r0, r1)
    C = T // Tc
    key = (round(r0, 9), round(r1, 9), Tc)
    if key not in _CACHE:
        _CACHE[key] = _build(r0, r1, Tc)
    nc = _CACHE[key]

    FREE = ML * Tc
    FCSZ = min(512, FREE)
    MPF = FCSZ // Tc
    tp = np.arange(Tc, dtype=np.float64)
    scale0 = ((1.0 - r0) * r0 ** -(tp + 1)).astype(np.float32)  # [Tc]
    rho64 = (1.0 - r1) * (r0 / r1) ** (tp + 1)
    rho_full = np.ascontiguousarray(
        np.broadcast_to(rho64.astype(np.float32), (128, ML, Tc))
        .reshape(128, ML * Tc))

    def prep_w(w):
        # [H, I] -> [p, ic, ht, hh] -> [128, KI*KH*128]
        return np.ascontiguousarray(
            np.asarray(w, np.float32).astype(BF)
            .reshape(KH, 128, KI, 128).transpose(3, 2, 0, 1).reshape(128, -1))

    w0p, w1p = prep_w(w_ih0), prep_w(w_ih1)
    # Bias via the scan's data1 operand: shift y_t = g_t - B_t turns the bias
    # add into data1 = -B_t (a constant). Layer-1 additionally absorbs the
    # bias-through-layer-0 term (rank-1: q = W1*b0) via the same shift.
    b0v = np.asarray(b_ih0, np.float64).ravel()
    b1v = np.asarray(b_ih1, np.float64).ravel()
    S0 = np.cumsum((1.0 - r0) * r0 ** -(tp + 1))          # [Tc]
    S1 = np.cumsum((1.0 - r1) * r1 ** -(tp + 1))
    w1b = np.asarray(w_ih1, np.float32).astype(BF).astype(np.float64)
    qv = w1b @ b0v                                        # [H]
    Pc = np.cumsum(rho64 * S0)                            # [Tc]
    B0g = b0v[None, :] * S0[:, None]                      # [Tc, H]
    corr1 = b1v[None, :] * S1[:, None] + qv[None, :] * Pc[:, None]  # [Tc, H]

    def to_bt(mat):  # [Tc, H] -> [128, KH*Tc] (p, ht, t)
        return np.ascontiguousarray(
            (-mat).T.reshape(KH, 128, Tc).transpose(1, 0, 2)
            .reshape(128, KH * Tc).astype(np.float32))

    bt0 = to_bt(B0g)
    bt1 = to_bt(corr1)
    ccp = np.empty((128, 2, KH), np.float32)
    ccp[:, 0, :] = (B0g[Tc - 1] * r0 ** Tc).reshape(KH, 128).T
    ccp[:, 1, :] = (corr1[Tc - 1] * r1 ** Tc).reshape(KH, 128).T

    input_ = np.asarray(input_, np.float32)
    h_0 = np.asarray(h_0, np.float32)

    in_maps = []
    for core in range(NCORES):
        xc = input_[:, core * ML:(core + 1) * ML, :]
        hc = h_0[:, core * ML:(core + 1) * ML, :]
        xp, h0T = _prep_core(xc, hc, scale0)
        in_maps.append({
            "x": xp, "w0": w0p, "w1": w1p,
            "bt0": bt0, "bt1": bt1, "cc": ccp,
            "rho": rho_full, "h0T": h0T,
        })

    res = run_bass_kernel_spmd(nc, in_maps, core_ids=list(range(NCORES)))
    results = res.results

    rpow1 = (r1 ** (tp + 1)).astype(np.float32)  # [Tc]
    corr_dev = corr1.T.reshape(KH, 128, Tc).astype(np.float32)  # [ht, p, t]
    all_h1 = np.empty((T, M, H), np.float32)
    h_n = np.empty((2, M, H), np.float32)
    for core in range(NCORES):
        g = results[core]["outg"].reshape(C, KH, 128, ML, Tc)
        g = g + corr_dev[None, :, :, None, :]
        hcore = (g * rpow1[None, None, None, None, :]) \
            .transpose(0, 4, 3, 1, 2).reshape(T, ML, H)
        all_h1[:, core * ML:(core + 1) * ML, :] = hcore
        hn0 = results[core]["outhn0"].reshape(128, KH, ML) \
            .transpose(2, 1, 0).reshape(ML, H)
        h_n[0, core * ML:(core + 1) * ML, :] = hn0
    h_n[1] = all_h1[T - 1]
    return all_h1, h_n
